# revision 11
# baseline (speedup 1.0000x reference)
"""Trainium2 Bass kernel for Mllama-style GQA self-attention (B=1, S=2048,
H=32 q-heads, KVH=8 kv-heads, D=128), tensor-parallel over heads across 8
NeuronCores.

Sharding: core c owns kv-head c and q-heads 4c..4c+3 (Wq/Wk/Wv column shards),
computes its heads' attention output in transposed [feature, seq] layout, then
computes the full-width partial output projection rows (Wo row shard, own 512
features contracted); the host sums the 8 cores' fp32 partials.

Matmuls run in fp16 (fp32 PSUM accumulation); the exp/probs path is bf16 for
range (scores reach ~17, exp overflows fp16). Attention runs on 256-query
chunks (exact block-causal) with the two heads of each GQA pair fused into
single 512-wide score/PV matmuls (one PSUM accumulation group per pair).
A quantum scheduler interleaves o_proj groups and the next projection chunk's
matmuls between each round's score and PV blocks, hiding the scalar-engine
exp latency so the in-order tensor queue never waits. The softmax normalize
uses a ones-matmul denominator + fast DVE reciprocal + ones-broadcast matmul,
pipelined across chunk boundaries.
"""
import math
from collections import deque
import numpy as np
import ml_dtypes

import concourse.bass as bass
import concourse.bacc as bacc
import concourse.mybir as mybir
import concourse.tile as tile
from concourse.bass_utils import run_bass_kernel_spmd

F16 = np.float16
BF16 = ml_dtypes.bfloat16
S, E, H, KVH, D = 2048, 4096, 32, 8, 128
N_CORES = 8
G = H // KVH                      # q heads per core (4)
NP = G // 2                       # head pairs per core (2)
OSH = G * D                       # per-core q/attn feature count (512)
PC = 512                          # projection seq chunk
N_PC = S // PC                    # 4
AC = 256                          # attention query chunk
AC2 = 2 * AC                      # paired width (512)
N_AC = S // AC                    # 8
NE = E // 128                     # 32 contraction tiles
N_ST = S // 128                   # 16 seq tiles
EXP_BIAS = -2.772588722239781     # -4*ln2: probs scaled 2^-4, cancels in norm

_BUILD_CACHE = {}


def build_bass(causal: bool):
    key = causal
    if key in _BUILD_CACHE:
        return _BUILD_CACHE[key]
    dt = mybir.dt
    nc = bacc.Bacc("TRN2", target_bir_lowering=False, debug=False,
                   enable_asserts=False, num_devices=N_CORES)

    XT4 = nc.dram_tensor("xt", [N_PC, 128, NE, PC], dt.float16, kind="ExternalInput").ap()
    WQT = nc.dram_tensor("wqt", [128, NE, OSH], dt.float16, kind="ExternalInput").ap()
    WKT = nc.dram_tensor("wkt", [128, NE, D], dt.float16, kind="ExternalInput").ap()
    WVT = nc.dram_tensor("wvt", [128, NE, D], dt.float16, kind="ExternalInput").ap()
    WOT = nc.dram_tensor("wot", [128, G, E], dt.float16, kind="ExternalInput").ap()
    ROPE = nc.dram_tensor("rope", [2, D, S], dt.float16, kind="ExternalInput").ap()
    TRI = nc.dram_tensor("tri", [2, 128, AC2], dt.bfloat16, kind="ExternalInput").ap()
    OUT = nc.dram_tensor("out", [S, E], dt.float32, kind="ExternalOutput").ap()

    with tile.TileContext(nc) as tc:
        with (
            tc.tile_pool(name="wpool", bufs=1) as wpool,
            tc.tile_pool(name="qkv", bufs=1) as qkvpool,
            tc.tile_pool(name="consts", bufs=1) as cpool,
            tc.tile_pool(name="xs", bufs=2) as xspool,
            tc.tile_pool(name="cs", bufs=2) as cspool,
            tc.tile_pool(name="rtmp", bufs=1) as rtmppool,
            tc.tile_pool(name="epool", bufs=3) as epool,
            tc.tile_pool(name="small", bufs=2) as smallpool,
            tc.tile_pool(name="attn", bufs=5) as attnpool,
            tc.tile_pool(name="outs", bufs=3) as outpool,
            tc.tile_pool(name="ps_qkv", bufs=2, space="PSUM") as ps_qkv,
            tc.tile_pool(name="ps_s", bufs=3, space="PSUM") as ps_s,
            tc.tile_pool(name="ps_ot", bufs=3, space="PSUM") as ps_ot,
        ):
            # --- resident weights, [128, NE, width]. Priority order: the
            # first chunk's activations and Wk jump ahead of the big weight
            # burst so the PE starts fast.
            wq_sb = wpool.tile([128, NE, OSH], dt.float16)
            wk_sb = wpool.tile([128, NE, D], dt.float16)
            wv_sb = wpool.tile([128, NE, D], dt.float16)
            wo_sb = wpool.tile([128, G, E], dt.float16)

            xs0 = xspool.tile([128, NE, PC], dt.float16, tag="xs")
            cs0 = cspool.tile([128, 2, PC], dt.float16, tag="cs")
            for q in range(8):
                nc.sync.dma_start(wk_sb[:, q * 4:(q + 1) * 4, :],
                                  WKT[:, q * 4:(q + 1) * 4, :])
                nc.sync.dma_start(xs0[:, q * 4:(q + 1) * 4, :],
                                  XT4[0, :, q * 4:(q + 1) * 4, :])
            for q in range(8):
                nc.sync.dma_start(wv_sb[:, q * 4:(q + 1) * 4, :],
                                  WVT[:, q * 4:(q + 1) * 4, :])
            nc.sync.dma_start(cs0[:], ROPE[:, :, 0:PC].rearrange("j p s -> p j s"))
            for q in range(8):
                nc.sync.dma_start(wq_sb[:, q * 4:(q + 1) * 4, :],
                                  WQT[:, q * 4:(q + 1) * 4, :])
            for q in range(4):
                nc.sync.dma_start(wo_sb[:, :, q * 1024:(q + 1) * 1024],
                                  WOT[:, :, q * 1024:(q + 1) * 1024])

            tri_sb = cpool.tile([128, 2, AC2], dt.bfloat16)
            nc.sync.dma_start(tri_sb[:], TRI.rearrange("j p f -> p j f"))
            ones_col = cpool.tile([128, 1], dt.bfloat16)
            nc.vector.memset(ones_col[:], 1.0)
            ones_row = cpool.tile([1, 128], dt.float16)
            nc.vector.memset(ones_row[:], 1.0)
            ebias = cpool.tile([128, 1], dt.float32)
            nc.vector.memset(ebias[:], EXP_BIAS)

            # --- persistent activations
            # qT: [d, chunk, head, within-chunk] so a head PAIR's queries for
            # one attention chunk are one contiguous 512-wide rhs.
            qT_sb = qkvpool.tile([128, N_AC, G, AC], dt.float16)
            kT_sb = qkvpool.tile([128, S], dt.float16)        # [d, s]
            v_sb = qkvpool.tile([128, N_ST, D], dt.bfloat16)  # per s-tile [t, d]

            # --- quantum scheduler state
            proj_q = deque()
            oproj_q = deque()

            def pump(n):
                for _ in range(n):
                    if proj_q:
                        proj_q.popleft()()
                    elif oproj_q:
                        oproj_q.popleft()()

            def flush(q):
                while q:
                    q.popleft()()

            # ---------- projection chunk -> quanta ----------
            def push_proj(sc):
                s0 = sc * PC
                if sc == 0:
                    xs, cs = xs0, cs0
                else:
                    xs = xspool.tile([128, NE, PC], dt.float16, tag="xs",
                                     name="xs")
                    for q in range(4):
                        nc.sync.dma_start(xs[:, q * 8:(q + 1) * 8, :],
                                          XT4[sc, :, q * 8:(q + 1) * 8, :])
                    cs = cspool.tile([128, 2, PC], dt.float16, tag="cs",
                                     name="cs")
                    nc.sync.dma_start(cs[:], ROPE[:, :, s0:s0 + PC]
                                      .rearrange("j p s -> p j s"))
                st8 = {}

                def rope_evac(hh, pq):
                    cos_t = cs[:, 0, :]
                    sin_t = cs[:, 1, :]
                    is_k = hh == G
                    rt = rtmppool.tile([128, 2, PC], dt.float32, tag="rt",
                                       name="rt")
                    t1, t2 = rt[:, 0, :], rt[:, 1, :]
                    nc.vector.tensor_mul(t1[0:64, :], pq[0:64, :], cos_t[0:64, :])
                    nc.vector.tensor_mul(t2[0:64, :], pq[64:128, :], sin_t[0:64, :])
                    nc.vector.tensor_mul(t1[64:128, :], pq[64:128, :], cos_t[64:128, :])
                    nc.vector.tensor_mul(t2[64:128, :], pq[0:64, :], sin_t[64:128, :])
                    if is_k:
                        nc.vector.tensor_add(kT_sb[:, s0:s0 + PC], t1[:], t2[:])
                    else:
                        for i in range(2):
                            nc.vector.tensor_add(
                                qT_sb[:, 2 * sc + i, hh, :],
                                t1[:, i * AC:(i + 1) * AC],
                                t2[:, i * AC:(i + 1) * AC])

                def q_quant(hh, part):   # part 0..7, 4 e-tiles each
                    def th():
                        if part == 0:
                            st8[hh] = ps_qkv.tile([128, PC], dt.float32,
                                                  tag="pq", name="pq")
                        pq = st8[hh]
                        is_k = hh == G
                        for e in range(part * 4, part * 4 + 4):
                            lhsT = (wk_sb[:, e, :] if is_k
                                    else wq_sb[:, e, hh * D:(hh + 1) * D])
                            nc.tensor.matmul(pq[:], lhsT, xs[:, e, :],
                                             start=(e == 0), stop=(e == NE - 1))
                        if part == 7:
                            rope_evac(hh, pq)
                    return th

                def v_quant(u, part):
                    def th():
                        if part == 0:
                            st8[('v', u)] = ps_qkv.tile([128, D], dt.float32,
                                                        tag="pq", name="pv")
                        pv = st8[('v', u)]
                        for e in range(part * 8, part * 8 + 8):
                            nc.tensor.matmul(pv[:],
                                             xs[:, e, u * 128:(u + 1) * 128],
                                             wv_sb[:, e, :],
                                             start=(e == 0), stop=(e == NE - 1))
                        if part == 3:
                            nc.vector.tensor_copy(v_sb[:, (s0 // 128) + u, :],
                                                  pv[:])
                    return th

                for part in range(8):
                    proj_q.append(q_quant(G, part))       # K first
                for u in range(PC // 128):
                    for part in range(4):
                        proj_q.append(v_quant(u, part))   # then V
                for hh in range(G):
                    for part in range(8):
                        proj_q.append(q_quant(hh, part))  # then Q heads

            # ---------- attention ----------
            attnTs = {c: {} for c in range(N_AC)}
            pendB = []          # (c, p, ot2, recip) awaiting stage B

            def push_oproj(c):
                ats = attnTs[c]
                c0 = c * AC

                def grp(t, pc8):
                    def th():
                        po = ps_qkv.tile([128, OSH], dt.float32, tag="pq",
                                         name="po")
                        for hh in range(G):
                            nc.tensor.matmul(
                                po[:],
                                ats[hh // 2][:, (hh % 2) * AC + t * 128:
                                             (hh % 2) * AC + (t + 1) * 128],
                                wo_sb[:, hh, pc8 * OSH:(pc8 + 1) * OSH],
                                start=(hh == 0), stop=(hh == G - 1))
                        o_sb = outpool.tile([128, OSH], dt.float32, tag="o")
                        if pc8 % 2 == 0:
                            nc.vector.tensor_copy(o_sb[:], po[:])
                        else:
                            nc.scalar.copy(o_sb[:], po[:])
                        nc.sync.dma_start(
                            OUT[c0 + t * 128: c0 + (t + 1) * 128,
                                pc8 * OSH:(pc8 + 1) * OSH], o_sb[:])
                    return th

                for t in range(AC // 128):
                    for pc8 in range(8):
                        oproj_q.append(grp(t, pc8))

            def stageB_flush():
                flush(oproj_q)
                pump(2)
                done = []
                for (c, p, ot2, recip) in pendB:
                    bc_ps = ps_s.tile([128, AC2], dt.float32, tag="st",
                                      name="bc")
                    nc.tensor.matmul(bc_ps[:], ones_row[:], recip[:],
                                     start=True, stop=True)
                    bc_sb = smallpool.tile([128, AC2], dt.float16, tag="bc_sb")
                    nc.scalar.copy(bc_sb[:], bc_ps[:])
                    attnT2 = attnpool.tile([128, AC2], dt.float16, tag="attnT")
                    nc.vector.tensor_mul(attnT2[:], ot2[:], bc_sb[:])
                    attnTs[c][p] = attnT2
                    done.append(c)
                pendB.clear()
                for c in sorted(set(done)):
                    push_oproj(c)

            def att_chunk(c):
                nb = 2 * (c + 1) if causal else N_ST
                d0 = 2 * c if causal else N_ST
                ots = [ps_ot.tile([128, AC2], dt.float32, tag="ot", name="ot")
                       for _ in range(NP)]
                esums = [epool.tile([128, AC2], dt.bfloat16, tag="esum",
                                    name="esum", bufs=2) for _ in range(NP)]
                for b in range(nb):
                    diag = causal and b >= d0
                    e2s = []
                    for p in range(NP):
                        st2 = ps_s.tile([128, AC2], dt.float32, tag="st",
                                        name="st")
                        nc.tensor.matmul(st2[:],
                                         kT_sb[:, b * 128:(b + 1) * 128],
                                         qT_sb[:, c, 2 * p:2 * p + 2, :],
                                         start=True, stop=True)
                        e2 = epool.tile([128, AC2], dt.bfloat16, tag="e",
                                        name="e2")
                        nc.scalar.activation(e2[:], st2[:],
                                             mybir.ActivationFunctionType.Exp,
                                             bias=ebias[:])
                        if diag:
                            nc.vector.tensor_mul(e2[:], e2[:],
                                                 tri_sb[:, b - d0, :])
                        e2s.append(e2)
                    pump(3 if diag else 2)
                    for p in range(NP):
                        nc.tensor.matmul(ots[p][:], v_sb[:, b, :], e2s[p][:],
                                         start=(b == 0), stop=(b == nb - 1))
                        with nc.allow_low_precision(reason="softmax denom"):
                            if b == 0:
                                nc.vector.tensor_copy(esums[p][:], e2s[p][:])
                            else:
                                nc.vector.tensor_add(esums[p][:], esums[p][:],
                                                     e2s[p][:])
                # stage A: denominator + fast reciprocal (off the PE path)
                for p in range(NP):
                    den = ps_s.tile([1, AC2], dt.float32, tag="st", name="den")
                    nc.tensor.matmul(den[:], ones_col[:], esums[p][:],
                                     start=True, stop=True)
                    recip = smallpool.tile([1, AC2], dt.float32, tag="recip")
                    nc.vector.reciprocal_approx_fast(recip[:], den[:])
                    recip16 = smallpool.tile([1, AC2], dt.float16,
                                             tag="recip16")
                    nc.scalar.copy(recip16[:], recip[:])
                    pendB.append((c, p, ots[p], recip16))

            # ===================== driver =====================
            push_proj(0)
            flush(proj_q)
            for c in range(N_AC):
                if c % 2 == 0 and c // 2 + 1 < N_PC:
                    push_proj(c // 2 + 1)
                stageB_flush()
                att_chunk(c)
                if c % 2 == 1:
                    flush(proj_q)
            stageB_flush()
            flush(oproj_q)

    nc.compile()
    _BUILD_CACHE[key] = nc
    return nc


def _prep_inputs(hidden_states, attention_mask, cos, sin, Wq, Wk, Wv, Wo):
    X = np.asarray(hidden_states, dtype=np.float32).reshape(S, E)
    # [N_PC, 128, NE, PC]: exact SBUF tile layout per chunk -> long DMA runs
    XT4 = np.ascontiguousarray(
        X.reshape(N_PC, PC, NE, 128).transpose(0, 3, 2, 1)).astype(F16)

    m = np.asarray(attention_mask, dtype=np.float32).reshape(S, S)
    il, ju = np.tril_indices(S), np.triu_indices(S, 1)
    causal = bool(np.all(m[il] == 0.0) and np.all(m[ju] <= -1e8))
    dense = bool(np.all(m == 0.0))
    if not (causal or dense):
        raise NotImplementedError("only causal or all-zero masks supported")

    # the 1/sqrt(D) score scale is folded into Wq host-side
    cosT = np.ascontiguousarray(np.asarray(cos, np.float32).reshape(S, D).T)
    sinT = np.ascontiguousarray(np.asarray(sin, np.float32).reshape(S, D).T)
    sin_mod = sinT.copy()
    sin_mod[0:64] *= -1.0
    rope_t = np.stack([cosT, sin_mod]).astype(F16)

    p = np.arange(128)[:, None]
    f = np.arange(AC)[None, :]
    tri1 = np.stack([(128 * jj + p <= f) for jj in range(2)])
    tri = np.concatenate([tri1, tri1], axis=2).astype(BF16)  # [2,128,2*AC]

    Wq = np.asarray(Wq, np.float32)
    Wk = np.asarray(Wk, np.float32)
    Wv = np.asarray(Wv, np.float32)
    Wo = np.asarray(Wo, np.float32)

    def wtile(Wshard, dtype=F16):
        # [out, E] -> SBUF layout [128, NE, out]
        return np.ascontiguousarray(
            Wshard.T.reshape(NE, 128, Wshard.shape[0]).transpose(1, 0, 2)
        ).astype(dtype)

    q_pre = 1.0 / math.sqrt(D)

    in_maps = []
    for c in range(N_CORES):
        in_maps.append({
            "xt": XT4,
            "wqt": wtile(Wq[c * OSH:(c + 1) * OSH, :] * q_pre),
            "wkt": wtile(Wk[c * D:(c + 1) * D, :]),
            "wvt": wtile(Wv[c * D:(c + 1) * D, :]),
            "wot": np.ascontiguousarray(
                Wo[:, c * OSH:(c + 1) * OSH].T.reshape(G, 128, E)
                .transpose(1, 0, 2)).astype(F16),
            "rope": rope_t,
            "tri": tri,
        })
    return in_maps, causal


def kernel(hidden_states, attention_mask, cos, sin, Wq, Wk, Wv, Wo,
           _trace=False, _tmpdir=None):
    in_maps, causal = _prep_inputs(hidden_states, attention_mask, cos, sin,
                                   Wq, Wk, Wv, Wo)
    nc = build_bass(causal)
    res = run_bass_kernel_spmd(nc, in_maps, core_ids=list(range(N_CORES)),
                               trace=_trace, tmpdir=_tmpdir)
    out = res.results[0]["out"].astype(np.float32)
    for c in range(1, N_CORES):
        out = out + res.results[c]["out"]
    kernel._last_result = res
    return out.reshape(1, S, E).astype(np.float32)


# revision 21
# speedup vs baseline: 1.0276x; 1.0276x over previous
"""Trainium2 Bass kernel for Mllama-style GQA self-attention (B=1, S=2048,
H=32 q-heads, KVH=8 kv-heads, D=128), tensor-parallel over heads across 8
NeuronCores.

Sharding: core c owns kv-head c and q-heads 4c..4c+3 (Wq/Wk/Wv column shards),
computes its heads' attention output in transposed [feature, seq] layout, then
computes the full-width partial output projection rows (Wo row shard, own 512
features contracted); the host sums the 8 cores' fp32 partials.

Matmuls run in fp16 (fp32 PSUM accumulation); the exp/probs path is bf16 for
range (scores reach ~17, exp overflows fp16). Attention runs on 256-query
chunks (exact block-causal) with the two heads of each GQA pair fused into
single 512-wide score/PV matmuls (one PSUM accumulation group per pair).
A quantum scheduler interleaves o_proj groups and the next projection chunk's
matmuls between each round's score and PV blocks, hiding the scalar-engine
exp latency so the in-order tensor queue never waits. The softmax normalize
uses a ones-matmul denominator + fast DVE reciprocal + ones-broadcast matmul,
pipelined across chunk boundaries.
"""
import math
from collections import deque
import numpy as np
import ml_dtypes

import concourse.bass as bass
import concourse.bacc as bacc
import concourse.mybir as mybir
import concourse.tile as tile
from concourse.bass_utils import run_bass_kernel_spmd

F16 = np.float16
BF16 = ml_dtypes.bfloat16
S, E, H, KVH, D = 2048, 4096, 32, 8, 128
N_CORES = 8
G = H // KVH                      # q heads per core (4)
NP = G // 2                       # head pairs per core (2)
OSH = G * D                       # per-core q/attn feature count (512)
PC = 512                          # projection seq chunk
N_PC = S // PC                    # 4
AC = 256                          # attention query chunk
AC2 = 2 * AC                      # paired width (512)
N_AC = S // AC                    # 8
NE = E // 128                     # 32 contraction tiles
N_ST = S // 128                   # 16 seq tiles
EXP_BIAS = -2.772588722239781     # -4*ln2: probs scaled 2^-4, cancels in norm

_BUILD_CACHE = {}


def build_bass(causal: bool):
    key = causal
    if key in _BUILD_CACHE:
        return _BUILD_CACHE[key]
    dt = mybir.dt
    nc = bacc.Bacc("TRN2", target_bir_lowering=False, debug=False,
                   enable_asserts=False, num_devices=N_CORES)

    XT4 = nc.dram_tensor("xt", [N_PC, 128, NE, PC], dt.float16, kind="ExternalInput").ap()
    WQT = nc.dram_tensor("wqt", [128, NE, OSH], dt.float16, kind="ExternalInput").ap()
    WKT = nc.dram_tensor("wkt", [128, NE, D], dt.float16, kind="ExternalInput").ap()
    WVT = nc.dram_tensor("wvt", [128, NE, D], dt.float16, kind="ExternalInput").ap()
    WOT = nc.dram_tensor("wot", [128, G, E], dt.float16, kind="ExternalInput").ap()
    ROPE = nc.dram_tensor("rope", [2, D, S], dt.float16, kind="ExternalInput").ap()
    TRI = nc.dram_tensor("tri", [3, 128, AC2], dt.bfloat16, kind="ExternalInput").ap()
    OUT = nc.dram_tensor("out", [S, E], dt.float16, kind="ExternalOutput").ap()

    with tile.TileContext(nc) as tc:
        with (
            tc.tile_pool(name="wpool", bufs=1) as wpool,
            tc.tile_pool(name="qkv", bufs=1) as qkvpool,
            tc.tile_pool(name="consts", bufs=1) as cpool,
            tc.tile_pool(name="xs", bufs=2) as xspool,
            tc.tile_pool(name="cs", bufs=2) as cspool,
            tc.tile_pool(name="rtmp", bufs=1) as rtmppool,
            tc.tile_pool(name="epool", bufs=3) as epool,
            tc.tile_pool(name="small", bufs=2) as smallpool,
            tc.tile_pool(name="attn", bufs=8) as attnpool,
            tc.tile_pool(name="outs", bufs=3) as outpool,
            tc.tile_pool(name="ps_qkv", bufs=3, space="PSUM") as ps_qkv,
            tc.tile_pool(name="ps_s", bufs=2, space="PSUM") as ps_s,
            tc.tile_pool(name="ps_ot", bufs=3, space="PSUM") as ps_ot,
        ):
            # --- resident weights, [128, NE, width]. Priority order: the
            # first chunk's activations and Wk jump ahead of the big weight
            # burst so the PE starts fast.
            wq_sb = wpool.tile([128, NE, OSH], dt.float16)
            wk_sb = wpool.tile([128, NE, D], dt.float16)
            wv_sb = wpool.tile([128, NE, D], dt.float16)
            wo_sb = wpool.tile([128, G, E], dt.float16)

            xs0 = xspool.tile([128, NE, PC], dt.float16, tag="xs")
            cs0 = cspool.tile([128, 2, PC], dt.float16, tag="cs")
            for q in range(8):
                nc.sync.dma_start(wk_sb[:, q * 4:(q + 1) * 4, :],
                                  WKT[:, q * 4:(q + 1) * 4, :])
                nc.sync.dma_start(xs0[:, q * 4:(q + 1) * 4, :],
                                  XT4[0, :, q * 4:(q + 1) * 4, :])
            for q in range(8):
                nc.sync.dma_start(wv_sb[:, q * 4:(q + 1) * 4, :],
                                  WVT[:, q * 4:(q + 1) * 4, :])
            nc.sync.dma_start(cs0[:], ROPE[:, :, 0:PC].rearrange("j p s -> p j s"))
            for q in range(8):
                nc.sync.dma_start(wq_sb[:, q * 4:(q + 1) * 4, :],
                                  WQT[:, q * 4:(q + 1) * 4, :])
            for q in range(4):
                nc.sync.dma_start(wo_sb[:, :, q * 1024:(q + 1) * 1024],
                                  WOT[:, :, q * 1024:(q + 1) * 1024])

            tri_sb = cpool.tile([128, 3, AC2], dt.bfloat16)
            nc.sync.dma_start(tri_sb[:], TRI.rearrange("j p f -> p j f"))
            ones_col = cpool.tile([128, 1], dt.bfloat16)
            nc.vector.memset(ones_col[:], 1.0)
            ones_row = cpool.tile([1, 128], dt.float16)
            nc.vector.memset(ones_row[:], 1.0)
            ebias = cpool.tile([128, 1], dt.float32)
            nc.vector.memset(ebias[:], EXP_BIAS)

            # --- persistent activations
            # qT: [d, chunk, head, within-chunk] so a head PAIR's queries for
            # one attention chunk are one contiguous 512-wide rhs.
            qT_sb = qkvpool.tile([128, N_AC, G, AC], dt.float16)
            kT_sb = qkvpool.tile([128, S], dt.float16)        # [d, s]
            v_sb = qkvpool.tile([128, N_ST, D], dt.bfloat16)  # per s-tile [t, d]

            # --- quantum scheduler state
            proj_q = deque()
            oproj_q = deque()

            def pump(n):
                for _ in range(n):
                    if proj_q:
                        proj_q.popleft()()
                    elif oproj_q:
                        oproj_q.popleft()()

            def flush(q):
                while q:
                    q.popleft()()

            # ---------- projection chunk -> quanta ----------
            def push_proj(sc):
                s0 = sc * PC
                if sc == 0:
                    xs, cs = xs0, cs0
                else:
                    xs = xspool.tile([128, NE, PC], dt.float16, tag="xs",
                                     name="xs")
                    for q in range(4):
                        nc.sync.dma_start(xs[:, q * 8:(q + 1) * 8, :],
                                          XT4[sc, :, q * 8:(q + 1) * 8, :])
                    cs = cspool.tile([128, 2, PC], dt.float16, tag="cs",
                                     name="cs")
                    nc.sync.dma_start(cs[:], ROPE[:, :, s0:s0 + PC]
                                      .rearrange("j p s -> p j s"))
                st8 = {}

                def rope_evac(hh, pq):
                    cos_t = cs[:, 0, :]
                    sin_t = cs[:, 1, :]
                    is_k = hh == G
                    rt = rtmppool.tile([128, 2, PC], dt.float32, tag="rt",
                                       name="rt")
                    t1, t2 = rt[:, 0, :], rt[:, 1, :]
                    nc.vector.tensor_mul(t1[0:64, :], pq[0:64, :], cos_t[0:64, :])
                    nc.vector.tensor_mul(t2[0:64, :], pq[64:128, :], sin_t[0:64, :])
                    nc.vector.tensor_mul(t1[64:128, :], pq[64:128, :], cos_t[64:128, :])
                    nc.vector.tensor_mul(t2[64:128, :], pq[0:64, :], sin_t[64:128, :])
                    if is_k:
                        nc.vector.tensor_add(kT_sb[:, s0:s0 + PC], t1[:], t2[:])
                    else:
                        for i in range(2):
                            nc.vector.tensor_add(
                                qT_sb[:, 2 * sc + i, hh, :],
                                t1[:, i * AC:(i + 1) * AC],
                                t2[:, i * AC:(i + 1) * AC])

                def q_quant(hh, part):   # part 0..7, 4 e-tiles each
                    def th():
                        if part == 0:
                            st8[hh] = ps_qkv.tile([128, PC], dt.float32,
                                                  tag="pq", name="pq")
                        pq = st8[hh]
                        is_k = hh == G
                        for e in range(part * 4, part * 4 + 4):
                            lhsT = (wk_sb[:, e, :] if is_k
                                    else wq_sb[:, e, hh * D:(hh + 1) * D])
                            nc.tensor.matmul(pq[:], lhsT, xs[:, e, :],
                                             start=(e == 0), stop=(e == NE - 1))
                        if part == 7:
                            rope_evac(hh, pq)
                    return th

                def v_quant(u, part):
                    def th():
                        if part == 0:
                            st8[('v', u)] = ps_qkv.tile([128, D], dt.float32,
                                                        tag="pq", name="pv")
                        pv = st8[('v', u)]
                        for e in range(part * 8, part * 8 + 8):
                            nc.tensor.matmul(pv[:],
                                             xs[:, e, u * 128:(u + 1) * 128],
                                             wv_sb[:, e, :],
                                             start=(e == 0), stop=(e == NE - 1))
                        if part == 3:
                            nc.vector.tensor_copy(v_sb[:, (s0 // 128) + u, :],
                                                  pv[:])
                    return th

                for part in range(8):
                    proj_q.append(q_quant(G, part))       # K first
                for u in range(PC // 128):
                    for part in range(4):
                        proj_q.append(v_quant(u, part))   # then V
                for hh in range(G):
                    for part in range(8):
                        proj_q.append(q_quant(hh, part))  # then Q heads

            # ---------- attention ----------
            attnTs = {}
            kmeta = {}
            pendB = []          # (key, p, ot2, recip) awaiting stage B

            def push_oproj(key):
                ats = attnTs[key]
                q0, qw = kmeta[key]

                def grp(t, pc8):
                    def th():
                        po = ps_qkv.tile([128, OSH], dt.float32, tag="pq",
                                         name="po")
                        for hh in range(G):
                            nc.tensor.matmul(
                                po[:],
                                ats[hh // 2][:, (hh % 2) * qw + t * 128:
                                             (hh % 2) * qw + (t + 1) * 128],
                                wo_sb[:, hh, pc8 * OSH:(pc8 + 1) * OSH],
                                start=(hh == 0), stop=(hh == G - 1))
                        o_sb = outpool.tile([128, OSH], dt.float16, tag="o")
                        if pc8 % 2 == 0:
                            nc.vector.tensor_copy(o_sb[:], po[:])
                        else:
                            nc.scalar.copy(o_sb[:], po[:])
                        nc.sync.dma_start(
                            OUT[q0 + t * 128: q0 + (t + 1) * 128,
                                pc8 * OSH:(pc8 + 1) * OSH], o_sb[:])
                    return th

                for t in range(qw // 128):
                    for pc8 in range(8):
                        oproj_q.append(grp(t, pc8))

            def stageB_flush():
                while len(oproj_q) > 8:
                    oproj_q.popleft()()
                pump(2)
                done = []
                for (key, p, ot2, recip) in pendB:
                    qw2 = 2 * kmeta[key][1]
                    sl = (lambda t: t[:]) if qw2 == AC2 \
                        else (lambda t: t[:, 0:qw2])
                    bc_ps = ps_s.tile([128, AC2], dt.float32, tag="st",
                                      name="bc")
                    rsl = recip[:] if qw2 == AC2 else recip[0:1, 0:qw2]
                    nc.tensor.matmul(sl(bc_ps), ones_row[:], rsl,
                                     start=True, stop=True)
                    bc_sb = smallpool.tile([128, AC2], dt.float16, tag="bc_sb")
                    nc.scalar.copy(sl(bc_sb), sl(bc_ps))
                    attnT2 = attnpool.tile([128, AC2], dt.float16, tag="attnT")
                    nc.vector.tensor_mul(sl(attnT2), sl(ot2), sl(bc_sb))
                    attnTs[key][p] = attnT2
                    done.append(key)
                pendB.clear()
                for key in dict.fromkeys(done):
                    if len(attnTs[key]) == NP:
                        push_oproj(key)

            def att_chunk(c, half=None):
                if half is None:
                    q0, qw = c * AC, AC
                    nb = 2 * (c + 1) if causal else N_ST
                    d0 = 2 * c if causal else N_ST
                else:
                    q0, qw = c * AC + half * 128, 128
                    nb = 2 * c + 1 + half
                    d0 = nb - 1
                key = (c, half)
                kmeta[key] = (q0, qw)
                attnTs[key] = {}
                qw2 = 2 * qw
                qoff = 0 if half is None else half * 128
                sl = (lambda t: t[:]) if qw2 == AC2 \
                    else (lambda t: t[:, 0:qw2])
                ots = [ps_ot.tile([128, AC2], dt.float32, tag="ot", name="ot")
                       for _ in range(NP)]
                esums = [epool.tile([128, AC2], dt.bfloat16, tag="esum",
                                    name="esum", bufs=2) for _ in range(NP)]
                for b in range(nb):
                    diag = causal and b >= d0
                    e2s = []
                    for p in range(NP):
                        st2 = ps_s.tile([128, AC2], dt.float32, tag="st",
                                        name="st")
                        nc.tensor.matmul(sl(st2),
                                         kT_sb[:, b * 128:(b + 1) * 128],
                                         qT_sb[:, c, 2 * p:2 * p + 2,
                                               qoff:qoff + qw],
                                         start=True, stop=True)
                        e2 = epool.tile([128, AC2], dt.bfloat16, tag="e",
                                        name="e2")
                        nc.scalar.activation(sl(e2), sl(st2),
                                             mybir.ActivationFunctionType.Exp,
                                             bias=ebias[:])
                        if diag:
                            tslot = 2 if half is not None else b - d0
                            nc.vector.tensor_mul(sl(e2), sl(e2),
                                                 tri_sb[:, tslot, 0:qw2])
                        e2s.append(e2)
                    pump(3 if diag else 2)
                    for p in range(NP):
                        nc.tensor.matmul(sl(ots[p]), v_sb[:, b, :],
                                         sl(e2s[p]),
                                         start=(b == 0), stop=(b == nb - 1))
                        with nc.allow_low_precision(reason="softmax denom"):
                            if b == 0:
                                nc.vector.tensor_copy(sl(esums[p]),
                                                      sl(e2s[p]))
                            else:
                                nc.vector.tensor_add(sl(esums[p]),
                                                     sl(esums[p]),
                                                     sl(e2s[p]))
                # stage A: denominator + fast reciprocal (off the PE path)
                for p in range(NP):
                    den = ps_s.tile([1, AC2], dt.float32, tag="st", name="den")
                    nc.tensor.matmul(den[0:1, 0:qw2] if qw2 != AC2 else den[:],
                                     ones_col[:], sl(esums[p]),
                                     start=True, stop=True)
                    recip = smallpool.tile([1, AC2], dt.float32, tag="recip")
                    nc.vector.reciprocal_approx_fast(
                        recip[0:1, 0:qw2] if qw2 != AC2 else recip[:],
                        den[0:1, 0:qw2] if qw2 != AC2 else den[:])
                    recip16 = smallpool.tile([1, AC2], dt.float16,
                                             tag="recip16")
                    nc.scalar.copy(
                        recip16[0:1, 0:qw2] if qw2 != AC2 else recip16[:],
                        recip[0:1, 0:qw2] if qw2 != AC2 else recip[:])
                    pendB.append((key, p, ots[p], recip16))

            # ===================== driver =====================
            push_proj(0)
            flush(proj_q)
            for c in range(N_AC):
                if c % 2 == 0 and c // 2 + 1 < N_PC:
                    push_proj(c // 2 + 1)
                stageB_flush()
                if causal and c == N_AC - 1:
                    att_chunk(c, half=0)
                    stageB_flush()
                    att_chunk(c, half=1)
                else:
                    att_chunk(c)
                if c % 2 == 1:
                    flush(proj_q)
            stageB_flush()
            flush(oproj_q)

    nc.compile()
    _BUILD_CACHE[key] = nc
    return nc


def _prep_inputs(hidden_states, attention_mask, cos, sin, Wq, Wk, Wv, Wo):
    X = np.asarray(hidden_states, dtype=np.float32).reshape(S, E)
    # [N_PC, 128, NE, PC]: exact SBUF tile layout per chunk -> long DMA runs
    XT4 = np.ascontiguousarray(
        X.reshape(N_PC, PC, NE, 128).transpose(0, 3, 2, 1)).astype(F16)

    m = np.asarray(attention_mask, dtype=np.float32).reshape(S, S)
    il, ju = np.tril_indices(S), np.triu_indices(S, 1)
    causal = bool(np.all(m[il] == 0.0) and np.all(m[ju] <= -1e8))
    dense = bool(np.all(m == 0.0))
    if not (causal or dense):
        raise NotImplementedError("only causal or all-zero masks supported")

    # the 1/sqrt(D) score scale is folded into Wq host-side
    cosT = np.ascontiguousarray(np.asarray(cos, np.float32).reshape(S, D).T)
    sinT = np.ascontiguousarray(np.asarray(sin, np.float32).reshape(S, D).T)
    sin_mod = sinT.copy()
    sin_mod[0:64] *= -1.0
    rope_t = np.stack([cosT, sin_mod]).astype(F16)

    p = np.arange(128)[:, None]
    f = np.arange(AC)[None, :]
    tri1 = np.stack([(128 * jj + p <= f) for jj in range(2)])
    tri12 = np.concatenate([tri1, tri1], axis=2)           # [2,128,2*AC]
    f128 = np.arange(128)[None, :]
    trih = np.tile((p <= f128), (1, 4))[None]              # [1,128,2*AC]
    tri = np.concatenate([tri12, trih], axis=0).astype(BF16)

    Wq = np.asarray(Wq, np.float32)
    Wk = np.asarray(Wk, np.float32)
    Wv = np.asarray(Wv, np.float32)
    Wo = np.asarray(Wo, np.float32)

    def wtile(Wshard, dtype=F16):
        # [out, E] -> SBUF layout [128, NE, out]
        return np.ascontiguousarray(
            Wshard.T.reshape(NE, 128, Wshard.shape[0]).transpose(1, 0, 2)
        ).astype(dtype)

    q_pre = 1.0 / math.sqrt(D)

    in_maps = []
    for c in range(N_CORES):
        in_maps.append({
            "xt": XT4,
            "wqt": wtile(Wq[c * OSH:(c + 1) * OSH, :] * q_pre),
            "wkt": wtile(Wk[c * D:(c + 1) * D, :]),
            "wvt": wtile(Wv[c * D:(c + 1) * D, :]),
            "wot": np.ascontiguousarray(
                Wo[:, c * OSH:(c + 1) * OSH].T.reshape(G, 128, E)
                .transpose(1, 0, 2)).astype(F16),
            "rope": rope_t,
            "tri": tri,
        })
    return in_maps, causal


def kernel(hidden_states, attention_mask, cos, sin, Wq, Wk, Wv, Wo,
           _trace=False, _tmpdir=None):
    in_maps, causal = _prep_inputs(hidden_states, attention_mask, cos, sin,
                                   Wq, Wk, Wv, Wo)
    nc = build_bass(causal)
    res = run_bass_kernel_spmd(nc, in_maps, core_ids=list(range(N_CORES)),
                               trace=_trace, tmpdir=_tmpdir)
    out = res.results[0]["out"].astype(np.float32)
    for c in range(1, N_CORES):
        out = out + res.results[c]["out"]
    kernel._last_result = res
    return out.reshape(1, S, E).astype(np.float32)


# revision 22
# speedup vs baseline: 1.0399x; 1.0120x over previous
"""Trainium2 Bass kernel for Mllama-style GQA self-attention (B=1, S=2048,
H=32 q-heads, KVH=8 kv-heads, D=128), tensor-parallel over heads across 8
NeuronCores.

Sharding: core c owns kv-head c and q-heads 4c..4c+3 (Wq/Wk/Wv column shards),
computes its heads' attention output in transposed [feature, seq] layout, then
computes the full-width partial output projection rows (Wo row shard, own 512
features contracted); the host sums the 8 cores' fp32 partials.

Matmuls run in fp16 (fp32 PSUM accumulation); the exp/probs path is bf16 for
range (scores reach ~17, exp overflows fp16). Attention runs on 256-query
chunks (exact block-causal) with the two heads of each GQA pair fused into
single 512-wide score/PV matmuls (one PSUM accumulation group per pair).
A quantum scheduler interleaves o_proj groups and the next projection chunk's
matmuls between each round's score and PV blocks, hiding the scalar-engine
exp latency so the in-order tensor queue never waits. The softmax normalize
uses a ones-matmul denominator + fast DVE reciprocal + ones-broadcast matmul,
pipelined across chunk boundaries.
"""
import math
from collections import deque
import numpy as np
import ml_dtypes

import concourse.bass as bass
import concourse.bacc as bacc
import concourse.mybir as mybir
import concourse.tile as tile
from concourse.bass_utils import run_bass_kernel_spmd

F16 = np.float16
BF16 = ml_dtypes.bfloat16
S, E, H, KVH, D = 2048, 4096, 32, 8, 128
N_CORES = 8
G = H // KVH                      # q heads per core (4)
NP = G // 2                       # head pairs per core (2)
OSH = G * D                       # per-core q/attn feature count (512)
PC = 512                          # projection seq chunk
N_PC = S // PC                    # 4
AC = 256                          # attention query chunk
AC2 = 2 * AC                      # paired width (512)
N_AC = S // AC                    # 8
NE = E // 128                     # 32 contraction tiles
N_ST = S // 128                   # 16 seq tiles
EXP_BIAS = -2.772588722239781     # -4*ln2: probs scaled 2^-4, cancels in norm

_BUILD_CACHE = {}


def build_bass(causal: bool):
    key = causal
    if key in _BUILD_CACHE:
        return _BUILD_CACHE[key]
    dt = mybir.dt
    nc = bacc.Bacc("TRN2", target_bir_lowering=False, debug=False,
                   enable_asserts=False, num_devices=N_CORES)

    XT4 = nc.dram_tensor("xt", [N_PC, 128, NE, PC], dt.float16, kind="ExternalInput").ap()
    WQT = nc.dram_tensor("wqt", [128, NE, OSH], dt.float16, kind="ExternalInput").ap()
    WKT = nc.dram_tensor("wkt", [128, NE, D], dt.float16, kind="ExternalInput").ap()
    WVT = nc.dram_tensor("wvt", [128, NE, D], dt.float16, kind="ExternalInput").ap()
    WOT = nc.dram_tensor("wot", [128, G, E], dt.float16, kind="ExternalInput").ap()
    ROPE = nc.dram_tensor("rope", [2, D, S], dt.float16, kind="ExternalInput").ap()
    TRI = nc.dram_tensor("tri", [3, 128, AC2], dt.bfloat16, kind="ExternalInput").ap()
    OUT = nc.dram_tensor("out", [S, E], dt.float16, kind="ExternalOutput").ap()

    with tile.TileContext(nc) as tc:
        with (
            tc.tile_pool(name="wpool", bufs=1) as wpool,
            tc.tile_pool(name="qkv", bufs=1) as qkvpool,
            tc.tile_pool(name="consts", bufs=1) as cpool,
            tc.tile_pool(name="xs", bufs=2) as xspool,
            tc.tile_pool(name="cs", bufs=2) as cspool,
            tc.tile_pool(name="rtmp", bufs=1) as rtmppool,
            tc.tile_pool(name="epool", bufs=3) as epool,
            tc.tile_pool(name="small", bufs=2) as smallpool,
            tc.tile_pool(name="attn", bufs=8) as attnpool,
            tc.tile_pool(name="outs", bufs=3) as outpool,
            tc.tile_pool(name="ps_qkv", bufs=3, space="PSUM") as ps_qkv,
            tc.tile_pool(name="ps_s", bufs=2, space="PSUM") as ps_s,
            tc.tile_pool(name="ps_ot", bufs=3, space="PSUM") as ps_ot,
        ):
            # --- resident weights, [128, NE, width]. Priority order: the
            # first chunk's activations and Wk jump ahead of the big weight
            # burst so the PE starts fast.
            wq_sb = wpool.tile([128, NE, OSH], dt.float16)
            wk_sb = wpool.tile([128, NE, D], dt.float16)
            wv_sb = wpool.tile([128, NE, D], dt.float16)
            wo_sb = wpool.tile([128, G, E], dt.float16)

            xs0 = xspool.tile([128, NE, PC], dt.float16, tag="xs")
            cs0 = cspool.tile([128, 2, PC], dt.float16, tag="cs")
            for q in range(8):
                nc.sync.dma_start(wk_sb[:, q * 4:(q + 1) * 4, :],
                                  WKT[:, q * 4:(q + 1) * 4, :])
                nc.sync.dma_start(xs0[:, q * 4:(q + 1) * 4, :],
                                  XT4[0, :, q * 4:(q + 1) * 4, :])
            for q in range(8):
                nc.sync.dma_start(wv_sb[:, q * 4:(q + 1) * 4, :],
                                  WVT[:, q * 4:(q + 1) * 4, :])
            nc.sync.dma_start(cs0[:], ROPE[:, :, 0:PC].rearrange("j p s -> p j s"))
            for q in range(8):
                nc.sync.dma_start(wq_sb[:, q * 4:(q + 1) * 4, :],
                                  WQT[:, q * 4:(q + 1) * 4, :])
            for q in range(4):
                nc.sync.dma_start(wo_sb[:, :, q * 1024:(q + 1) * 1024],
                                  WOT[:, :, q * 1024:(q + 1) * 1024])

            tri_sb = cpool.tile([128, 3, AC2], dt.bfloat16)
            nc.sync.dma_start(tri_sb[:], TRI.rearrange("j p f -> p j f"))
            ones_col = cpool.tile([128, 1], dt.bfloat16)
            nc.vector.memset(ones_col[:], 1.0)
            ones_row = cpool.tile([1, 128], dt.float16)
            nc.vector.memset(ones_row[:], 1.0)
            ebias = cpool.tile([128, 1], dt.float32)
            nc.vector.memset(ebias[:], EXP_BIAS)

            # --- persistent activations
            # qT: [d, chunk, head, within-chunk] so a head PAIR's queries for
            # one attention chunk are one contiguous 512-wide rhs.
            qT_sb = qkvpool.tile([128, N_AC, G, AC], dt.float16)
            kT_sb = qkvpool.tile([128, S], dt.float16)        # [d, s]
            v_sb = qkvpool.tile([128, N_ST, D], dt.bfloat16)  # per s-tile [t, d]

            # --- quantum scheduler state
            proj_q = deque()
            oproj_q = deque()

            def pump(n):
                for _ in range(n):
                    if proj_q:
                        proj_q.popleft()()
                    elif oproj_q:
                        oproj_q.popleft()()

            def flush(q):
                while q:
                    q.popleft()()

            # ---------- projection chunk -> quanta ----------
            def push_proj(sc):
                s0 = sc * PC
                if sc == 0:
                    xs, cs = xs0, cs0
                else:
                    xs = xspool.tile([128, NE, PC], dt.float16, tag="xs",
                                     name="xs")
                    for q in range(4):
                        nc.sync.dma_start(xs[:, q * 8:(q + 1) * 8, :],
                                          XT4[sc, :, q * 8:(q + 1) * 8, :])
                    cs = cspool.tile([128, 2, PC], dt.float16, tag="cs",
                                     name="cs")
                    nc.sync.dma_start(cs[:], ROPE[:, :, s0:s0 + PC]
                                      .rearrange("j p s -> p j s"))
                st8 = {}

                def rope_evac(hh, pq):
                    cos_t = cs[:, 0, :]
                    sin_t = cs[:, 1, :]
                    is_k = hh == G
                    rt = rtmppool.tile([128, 2, PC], dt.float32, tag="rt",
                                       name="rt")
                    t1, t2 = rt[:, 0, :], rt[:, 1, :]
                    nc.vector.tensor_mul(t1[0:64, :], pq[0:64, :], cos_t[0:64, :])
                    nc.vector.tensor_mul(t2[0:64, :], pq[64:128, :], sin_t[0:64, :])
                    nc.vector.tensor_mul(t1[64:128, :], pq[64:128, :], cos_t[64:128, :])
                    nc.vector.tensor_mul(t2[64:128, :], pq[0:64, :], sin_t[64:128, :])
                    if is_k:
                        nc.vector.tensor_add(kT_sb[:, s0:s0 + PC], t1[:], t2[:])
                    else:
                        for i in range(2):
                            nc.vector.tensor_add(
                                qT_sb[:, 2 * sc + i, hh, :],
                                t1[:, i * AC:(i + 1) * AC],
                                t2[:, i * AC:(i + 1) * AC])

                def q_quant(hh, part):   # part 0..7, 4 e-tiles each
                    def th():
                        if part == 0:
                            st8[hh] = ps_qkv.tile([128, PC], dt.float32,
                                                  tag="pq", name="pq")
                        pq = st8[hh]
                        is_k = hh == G
                        for e in range(part * 4, part * 4 + 4):
                            lhsT = (wk_sb[:, e, :] if is_k
                                    else wq_sb[:, e, hh * D:(hh + 1) * D])
                            nc.tensor.matmul(pq[:], lhsT, xs[:, e, :],
                                             start=(e == 0), stop=(e == NE - 1))
                        if part == 7:
                            rope_evac(hh, pq)
                    return th

                def v_quant(u, part):
                    def th():
                        if part == 0:
                            st8[('v', u)] = ps_qkv.tile([128, D], dt.float32,
                                                        tag="pq", name="pv")
                        pv = st8[('v', u)]
                        for e in range(part * 8, part * 8 + 8):
                            nc.tensor.matmul(pv[:],
                                             xs[:, e, u * 128:(u + 1) * 128],
                                             wv_sb[:, e, :],
                                             start=(e == 0), stop=(e == NE - 1))
                        if part == 3:
                            nc.vector.tensor_copy(v_sb[:, (s0 // 128) + u, :],
                                                  pv[:])
                    return th

                for part in range(8):
                    proj_q.append(q_quant(G, part))       # K first
                for u in range(PC // 128):
                    for part in range(4):
                        proj_q.append(v_quant(u, part))   # then V
                for hh in range(G):
                    for part in range(8):
                        proj_q.append(q_quant(hh, part))  # then Q heads

            # ---------- attention ----------
            attnTs = {}
            kmeta = {}
            pendB = []          # (key, p, ot2, recip) awaiting stage B

            def push_oproj(key):
                ats = attnTs[key]
                q0, qw = kmeta[key]

                def grp(t, pc8):
                    def th():
                        po = ps_qkv.tile([128, OSH], dt.float32, tag="pq",
                                         name="po")
                        for hh in range(G):
                            nc.tensor.matmul(
                                po[:],
                                ats[hh // 2][:, (hh % 2) * qw + t * 128:
                                             (hh % 2) * qw + (t + 1) * 128],
                                wo_sb[:, hh, pc8 * OSH:(pc8 + 1) * OSH],
                                start=(hh == 0), stop=(hh == G - 1))
                        o_sb = outpool.tile([128, OSH], dt.float16, tag="o")
                        if pc8 % 2 == 0:
                            nc.vector.tensor_copy(o_sb[:], po[:])
                        else:
                            nc.scalar.copy(o_sb[:], po[:])
                        nc.sync.dma_start(
                            OUT[q0 + t * 128: q0 + (t + 1) * 128,
                                pc8 * OSH:(pc8 + 1) * OSH], o_sb[:])
                    return th

                for t in range(qw // 128):
                    for pc8 in range(8):
                        oproj_q.append(grp(t, pc8))

            def stageB_flush():
                while len(oproj_q) > 8:
                    oproj_q.popleft()()
                pump(2)
                done = []
                for (key, p, ot2, recip) in pendB:
                    qw2 = 2 * kmeta[key][1]
                    sl = (lambda t: t[:]) if qw2 == AC2 \
                        else (lambda t: t[:, 0:qw2])
                    bc_ps = ps_s.tile([128, AC2], dt.float32, tag="st",
                                      name="bc")
                    rsl = recip[:] if qw2 == AC2 else recip[0:1, 0:qw2]
                    nc.tensor.matmul(sl(bc_ps), ones_row[:], rsl,
                                     start=True, stop=True)
                    bc_sb = smallpool.tile([128, AC2], dt.float16, tag="bc_sb")
                    nc.scalar.copy(sl(bc_sb), sl(bc_ps))
                    attnT2 = attnpool.tile([128, AC2], dt.float16, tag="attnT")
                    nc.vector.tensor_mul(sl(attnT2), sl(ot2), sl(bc_sb))
                    attnTs[key][p] = attnT2
                    done.append(key)
                pendB.clear()
                for key in dict.fromkeys(done):
                    if len(attnTs[key]) == NP:
                        push_oproj(key)

            def att_chunk(c, half=None, last=False):
                if half is None:
                    q0, qw = c * AC, AC
                    nb = 2 * (c + 1) if causal else N_ST
                    d0 = 2 * c if causal else N_ST
                else:
                    q0, qw = c * AC + half * 128, 128
                    nb = 2 * c + 1 + half
                    d0 = nb - 1
                key = (c, half)
                kmeta[key] = (q0, qw)
                attnTs[key] = {}
                qw2 = 2 * qw
                qoff = 0 if half is None else half * 128
                sl = (lambda t: t[:]) if qw2 == AC2 \
                    else (lambda t: t[:, 0:qw2])
                ots = [ps_ot.tile([128, AC2], dt.float32, tag="ot", name="ot")
                       for _ in range(NP)]
                esums = [epool.tile([128, AC2], dt.bfloat16, tag="esum",
                                    name="esum", bufs=2) for _ in range(NP)]
                for b in range(nb):
                    diag = causal and b >= d0
                    e2s = []
                    for p in range(NP):
                        st2 = ps_s.tile([128, AC2], dt.float32, tag="st",
                                        name="st")
                        nc.tensor.matmul(sl(st2),
                                         kT_sb[:, b * 128:(b + 1) * 128],
                                         qT_sb[:, c, 2 * p:2 * p + 2,
                                               qoff:qoff + qw],
                                         start=True, stop=True)
                        e2 = epool.tile([128, AC2], dt.bfloat16, tag="e",
                                        name="e2")
                        nc.scalar.activation(sl(e2), sl(st2),
                                             mybir.ActivationFunctionType.Exp,
                                             bias=ebias[:])
                        if diag:
                            tslot = 2 if half is not None else b - d0
                            nc.vector.tensor_mul(sl(e2), sl(e2),
                                                 tri_sb[:, tslot, 0:qw2])
                        e2s.append(e2)
                    pump(4 if last else (3 if diag else 2))
                    for p in range(NP):
                        nc.tensor.matmul(sl(ots[p]), v_sb[:, b, :],
                                         sl(e2s[p]),
                                         start=(b == 0), stop=(b == nb - 1))
                        with nc.allow_low_precision(reason="softmax denom"):
                            if b == 0:
                                nc.vector.tensor_copy(sl(esums[p]),
                                                      sl(e2s[p]))
                            else:
                                nc.vector.tensor_add(sl(esums[p]),
                                                     sl(esums[p]),
                                                     sl(e2s[p]))
                # stage A: denominator + fast reciprocal (off the PE path)
                for p in range(NP):
                    den = ps_s.tile([1, AC2], dt.float32, tag="st", name="den")
                    nc.tensor.matmul(den[0:1, 0:qw2] if qw2 != AC2 else den[:],
                                     ones_col[:], sl(esums[p]),
                                     start=True, stop=True)
                    recip = smallpool.tile([1, AC2], dt.float32, tag="recip")
                    nc.vector.reciprocal_approx_fast(
                        recip[0:1, 0:qw2] if qw2 != AC2 else recip[:],
                        den[0:1, 0:qw2] if qw2 != AC2 else den[:])
                    recip16 = smallpool.tile([1, AC2], dt.float16,
                                             tag="recip16")
                    nc.scalar.copy(
                        recip16[0:1, 0:qw2] if qw2 != AC2 else recip16[:],
                        recip[0:1, 0:qw2] if qw2 != AC2 else recip[:])
                    pendB.append((key, p, ots[p], recip16))

            # ===================== driver =====================
            push_proj(0)
            flush(proj_q)
            for c in range(N_AC):
                if c % 2 == 0 and c // 2 + 1 < N_PC:
                    push_proj(c // 2 + 1)
                stageB_flush()
                if causal and c == N_AC - 1:
                    att_chunk(c, half=0, last=True)
                    stageB_flush()
                    att_chunk(c, half=1, last=True)
                else:
                    att_chunk(c)
                if c % 2 == 1:
                    flush(proj_q)
            stageB_flush()
            flush(oproj_q)

    nc.compile()
    _BUILD_CACHE[key] = nc
    return nc


def _prep_inputs(hidden_states, attention_mask, cos, sin, Wq, Wk, Wv, Wo):
    X = np.asarray(hidden_states, dtype=np.float32).reshape(S, E)
    # [N_PC, 128, NE, PC]: exact SBUF tile layout per chunk -> long DMA runs
    XT4 = np.ascontiguousarray(
        X.reshape(N_PC, PC, NE, 128).transpose(0, 3, 2, 1)).astype(F16)

    m = np.asarray(attention_mask, dtype=np.float32).reshape(S, S)
    il, ju = np.tril_indices(S), np.triu_indices(S, 1)
    causal = bool(np.all(m[il] == 0.0) and np.all(m[ju] <= -1e8))
    dense = bool(np.all(m == 0.0))
    if not (causal or dense):
        raise NotImplementedError("only causal or all-zero masks supported")

    # the 1/sqrt(D) score scale is folded into Wq host-side
    cosT = np.ascontiguousarray(np.asarray(cos, np.float32).reshape(S, D).T)
    sinT = np.ascontiguousarray(np.asarray(sin, np.float32).reshape(S, D).T)
    sin_mod = sinT.copy()
    sin_mod[0:64] *= -1.0
    rope_t = np.stack([cosT, sin_mod]).astype(F16)

    p = np.arange(128)[:, None]
    f = np.arange(AC)[None, :]
    tri1 = np.stack([(128 * jj + p <= f) for jj in range(2)])
    tri12 = np.concatenate([tri1, tri1], axis=2)           # [2,128,2*AC]
    f128 = np.arange(128)[None, :]
    trih = np.tile((p <= f128), (1, 4))[None]              # [1,128,2*AC]
    tri = np.concatenate([tri12, trih], axis=0).astype(BF16)

    Wq = np.asarray(Wq, np.float32)
    Wk = np.asarray(Wk, np.float32)
    Wv = np.asarray(Wv, np.float32)
    Wo = np.asarray(Wo, np.float32)

    def wtile(Wshard, dtype=F16):
        # [out, E] -> SBUF layout [128, NE, out]
        return np.ascontiguousarray(
            Wshard.T.reshape(NE, 128, Wshard.shape[0]).transpose(1, 0, 2)
        ).astype(dtype)

    q_pre = 1.0 / math.sqrt(D)

    in_maps = []
    for c in range(N_CORES):
        in_maps.append({
            "xt": XT4,
            "wqt": wtile(Wq[c * OSH:(c + 1) * OSH, :] * q_pre),
            "wkt": wtile(Wk[c * D:(c + 1) * D, :]),
            "wvt": wtile(Wv[c * D:(c + 1) * D, :]),
            "wot": np.ascontiguousarray(
                Wo[:, c * OSH:(c + 1) * OSH].T.reshape(G, 128, E)
                .transpose(1, 0, 2)).astype(F16),
            "rope": rope_t,
            "tri": tri,
        })
    return in_maps, causal


def kernel(hidden_states, attention_mask, cos, sin, Wq, Wk, Wv, Wo,
           _trace=False, _tmpdir=None):
    in_maps, causal = _prep_inputs(hidden_states, attention_mask, cos, sin,
                                   Wq, Wk, Wv, Wo)
    nc = build_bass(causal)
    res = run_bass_kernel_spmd(nc, in_maps, core_ids=list(range(N_CORES)),
                               trace=_trace, tmpdir=_tmpdir)
    out = res.results[0]["out"].astype(np.float32)
    for c in range(1, N_CORES):
        out = out + res.results[c]["out"]
    kernel._last_result = res
    return out.reshape(1, S, E).astype(np.float32)


# revision 23
# speedup vs baseline: 1.0617x; 1.0210x over previous
"""Trainium2 Bass kernel for Mllama-style GQA self-attention (B=1, S=2048,
H=32 q-heads, KVH=8 kv-heads, D=128), tensor-parallel over heads across 8
NeuronCores.

Sharding: core c owns kv-head c and q-heads 4c..4c+3 (Wq/Wk/Wv column shards),
computes its heads' attention output in transposed [feature, seq] layout, then
computes the full-width partial output projection rows (Wo row shard, own 512
features contracted); the host sums the 8 cores' fp32 partials.

Matmuls run in fp16 (fp32 PSUM accumulation); the exp/probs path is bf16 for
range (scores reach ~17, exp overflows fp16). Attention runs on 256-query
chunks (exact block-causal) with the two heads of each GQA pair fused into
single 512-wide score/PV matmuls (one PSUM accumulation group per pair).
A quantum scheduler interleaves o_proj groups and the next projection chunk's
matmuls between each round's score and PV blocks, hiding the scalar-engine
exp latency so the in-order tensor queue never waits. The softmax normalize
uses a ones-matmul denominator + fast DVE reciprocal + ones-broadcast matmul,
pipelined across chunk boundaries.
"""
import math
from collections import deque
import numpy as np
import ml_dtypes

import concourse.bass as bass
import concourse.bacc as bacc
import concourse.mybir as mybir
import concourse.tile as tile
from concourse.bass_utils import run_bass_kernel_spmd

F16 = np.float16
BF16 = ml_dtypes.bfloat16
S, E, H, KVH, D = 2048, 4096, 32, 8, 128
N_CORES = 8
G = H // KVH                      # q heads per core (4)
NP = G // 2                       # head pairs per core (2)
OSH = G * D                       # per-core q/attn feature count (512)
PC = 512                          # projection seq chunk
N_PC = S // PC                    # 4
AC = 256                          # attention query chunk
AC2 = 2 * AC                      # paired width (512)
N_AC = S // AC                    # 8
NE = E // 128                     # 32 contraction tiles
N_ST = S // 128                   # 16 seq tiles
EXP_BIAS = -2.772588722239781     # -4*ln2: probs scaled 2^-4, cancels in norm

_BUILD_CACHE = {}


def build_bass(causal: bool):
    key = causal
    if key in _BUILD_CACHE:
        return _BUILD_CACHE[key]
    dt = mybir.dt
    nc = bacc.Bacc("TRN2", target_bir_lowering=False, debug=False,
                   enable_asserts=False, num_devices=N_CORES)

    XT4 = nc.dram_tensor("xt", [N_PC, 128, NE, PC], dt.float16, kind="ExternalInput").ap()
    WQT = nc.dram_tensor("wqt", [128, NE, OSH], dt.float16, kind="ExternalInput").ap()
    WKT = nc.dram_tensor("wkt", [128, NE, D], dt.float16, kind="ExternalInput").ap()
    WVT = nc.dram_tensor("wvt", [128, NE, D], dt.float16, kind="ExternalInput").ap()
    WOT = nc.dram_tensor("wot", [128, G, E], dt.float16, kind="ExternalInput").ap()
    ROPE = nc.dram_tensor("rope", [2, D, S], dt.float16, kind="ExternalInput").ap()
    TRI = nc.dram_tensor("tri", [3, 128, AC2], dt.bfloat16, kind="ExternalInput").ap()
    OUT = nc.dram_tensor("out", [S, E], dt.float16, kind="ExternalOutput").ap()

    with tile.TileContext(nc) as tc:
        with (
            tc.tile_pool(name="wpool", bufs=1) as wpool,
            tc.tile_pool(name="qkv", bufs=1) as qkvpool,
            tc.tile_pool(name="consts", bufs=1) as cpool,
            tc.tile_pool(name="xs", bufs=2) as xspool,
            tc.tile_pool(name="cs", bufs=2) as cspool,
            tc.tile_pool(name="rtmp", bufs=1) as rtmppool,
            tc.tile_pool(name="epool", bufs=3) as epool,
            tc.tile_pool(name="small", bufs=2) as smallpool,
            tc.tile_pool(name="attn", bufs=8) as attnpool,
            tc.tile_pool(name="outs", bufs=4) as outpool,
            tc.tile_pool(name="ps_qkv", bufs=3, space="PSUM") as ps_qkv,
            tc.tile_pool(name="ps_s", bufs=2, space="PSUM") as ps_s,
            tc.tile_pool(name="ps_ot", bufs=3, space="PSUM") as ps_ot,
        ):
            # --- resident weights, [128, NE, width]. Priority order: the
            # first chunk's activations and Wk jump ahead of the big weight
            # burst so the PE starts fast.
            wq_sb = wpool.tile([128, NE, OSH], dt.float16)
            wk_sb = wpool.tile([128, NE, D], dt.float16)
            wv_sb = wpool.tile([128, NE, D], dt.float16)
            wo_sb = wpool.tile([128, G, E], dt.float16)

            xs0 = xspool.tile([128, NE, PC], dt.float16, tag="xs")
            cs0 = cspool.tile([128, 2, PC], dt.float16, tag="cs")
            for q in range(8):
                nc.sync.dma_start(wk_sb[:, q * 4:(q + 1) * 4, :],
                                  WKT[:, q * 4:(q + 1) * 4, :])
                nc.sync.dma_start(xs0[:, q * 4:(q + 1) * 4, :],
                                  XT4[0, :, q * 4:(q + 1) * 4, :])
            for q in range(8):
                nc.sync.dma_start(wv_sb[:, q * 4:(q + 1) * 4, :],
                                  WVT[:, q * 4:(q + 1) * 4, :])
            nc.sync.dma_start(cs0[:], ROPE[:, :, 0:PC].rearrange("j p s -> p j s"))
            for q in range(8):
                nc.sync.dma_start(wq_sb[:, q * 4:(q + 1) * 4, :],
                                  WQT[:, q * 4:(q + 1) * 4, :])
            for q in range(4):
                nc.sync.dma_start(wo_sb[:, :, q * 1024:(q + 1) * 1024],
                                  WOT[:, :, q * 1024:(q + 1) * 1024])

            tri_sb = cpool.tile([128, 3, AC2], dt.bfloat16)
            nc.sync.dma_start(tri_sb[:], TRI.rearrange("j p f -> p j f"))
            ones_col = cpool.tile([128, 1], dt.bfloat16)
            nc.vector.memset(ones_col[:], 1.0)
            ones_row = cpool.tile([1, 128], dt.float16)
            nc.vector.memset(ones_row[:], 1.0)
            ebias = cpool.tile([128, 1], dt.float32)
            nc.vector.memset(ebias[:], EXP_BIAS)

            # --- persistent activations
            # qT: [d, chunk, head, within-chunk] so a head PAIR's queries for
            # one attention chunk are one contiguous 512-wide rhs.
            qT_sb = qkvpool.tile([128, N_AC, G, AC], dt.float16)
            kT_sb = qkvpool.tile([128, S], dt.float16)        # [d, s]
            v_sb = qkvpool.tile([128, N_ST, D], dt.bfloat16)  # per s-tile [t, d]

            # --- quantum scheduler state
            proj_q = deque()
            oproj_q = deque()

            def pump(n):
                for _ in range(n):
                    if proj_q:
                        proj_q.popleft()()
                    elif oproj_q:
                        oproj_q.popleft()()

            def flush(q):
                while q:
                    q.popleft()()

            # ---------- projection chunk -> quanta ----------
            def push_proj(sc):
                s0 = sc * PC
                if sc == 0:
                    xs, cs = xs0, cs0
                else:
                    xs = xspool.tile([128, NE, PC], dt.float16, tag="xs",
                                     name="xs")
                    for q in range(4):
                        nc.sync.dma_start(xs[:, q * 8:(q + 1) * 8, :],
                                          XT4[sc, :, q * 8:(q + 1) * 8, :])
                    cs = cspool.tile([128, 2, PC], dt.float16, tag="cs",
                                     name="cs")
                    nc.sync.dma_start(cs[:], ROPE[:, :, s0:s0 + PC]
                                      .rearrange("j p s -> p j s"))
                st8 = {}

                def rope_evac(hh, pq):
                    cos_t = cs[:, 0, :]
                    sin_t = cs[:, 1, :]
                    is_k = hh == G
                    rt = rtmppool.tile([128, 2, PC], dt.float32, tag="rt",
                                       name="rt")
                    t1, t2 = rt[:, 0, :], rt[:, 1, :]
                    nc.vector.tensor_mul(t1[0:64, :], pq[0:64, :], cos_t[0:64, :])
                    nc.vector.tensor_mul(t2[0:64, :], pq[64:128, :], sin_t[0:64, :])
                    nc.vector.tensor_mul(t1[64:128, :], pq[64:128, :], cos_t[64:128, :])
                    nc.vector.tensor_mul(t2[64:128, :], pq[0:64, :], sin_t[64:128, :])
                    if is_k:
                        nc.vector.tensor_add(kT_sb[:, s0:s0 + PC], t1[:], t2[:])
                    else:
                        for i in range(2):
                            nc.vector.tensor_add(
                                qT_sb[:, 2 * sc + i, hh, :],
                                t1[:, i * AC:(i + 1) * AC],
                                t2[:, i * AC:(i + 1) * AC])

                def q_quant(hh, part):   # part 0..7, 4 e-tiles each
                    def th():
                        if part == 0:
                            st8[hh] = ps_qkv.tile([128, PC], dt.float32,
                                                  tag="pq", name="pq")
                        pq = st8[hh]
                        is_k = hh == G
                        for e in range(part * 4, part * 4 + 4):
                            lhsT = (wk_sb[:, e, :] if is_k
                                    else wq_sb[:, e, hh * D:(hh + 1) * D])
                            nc.tensor.matmul(pq[:], lhsT, xs[:, e, :],
                                             start=(e == 0), stop=(e == NE - 1))
                        if part == 7:
                            rope_evac(hh, pq)
                    return th

                def v_quant(u, part):
                    def th():
                        if part == 0:
                            st8[('v', u)] = ps_qkv.tile([128, D], dt.float32,
                                                        tag="pq", name="pv")
                        pv = st8[('v', u)]
                        for e in range(part * 8, part * 8 + 8):
                            nc.tensor.matmul(pv[:],
                                             xs[:, e, u * 128:(u + 1) * 128],
                                             wv_sb[:, e, :],
                                             start=(e == 0), stop=(e == NE - 1))
                        if part == 3:
                            nc.vector.tensor_copy(v_sb[:, (s0 // 128) + u, :],
                                                  pv[:])
                    return th

                for part in range(8):
                    proj_q.append(q_quant(G, part))       # K first
                for u in range(PC // 128):
                    for part in range(4):
                        proj_q.append(v_quant(u, part))   # then V
                for hh in range(G):
                    for part in range(8):
                        proj_q.append(q_quant(hh, part))  # then Q heads

            # ---------- attention ----------
            attnTs = {}
            kmeta = {}
            pendB = []          # (key, p, ot2, recip) awaiting stage B

            def push_oproj(key):
                ats = attnTs[key]
                q0, qw = kmeta[key]

                def grp(t, pc8):
                    def th():
                        po = ps_qkv.tile([128, OSH], dt.float32, tag="pq",
                                         name="po")
                        for hh in range(G):
                            nc.tensor.matmul(
                                po[:],
                                ats[hh // 2][:, (hh % 2) * qw + t * 128:
                                             (hh % 2) * qw + (t + 1) * 128],
                                wo_sb[:, hh, pc8 * OSH:(pc8 + 1) * OSH],
                                start=(hh == 0), stop=(hh == G - 1))
                        o_sb = outpool.tile([128, OSH], dt.float16, tag="o")
                        if pc8 % 2 == 0:
                            nc.vector.tensor_copy(o_sb[:], po[:])
                        else:
                            nc.scalar.copy(o_sb[:], po[:])
                        nc.sync.dma_start(
                            OUT[q0 + t * 128: q0 + (t + 1) * 128,
                                pc8 * OSH:(pc8 + 1) * OSH], o_sb[:])
                    return th

                for t in range(qw // 128):
                    for pc8 in range(8):
                        oproj_q.append(grp(t, pc8))

            def stageB_flush():
                while len(oproj_q) > 8:
                    oproj_q.popleft()()
                pump(2)
                done = []
                for (key, p, ot2, recip) in pendB:
                    qw2 = 2 * kmeta[key][1]
                    sl = (lambda t: t[:]) if qw2 == AC2 \
                        else (lambda t: t[:, 0:qw2])
                    bc_ps = ps_s.tile([128, AC2], dt.float32, tag="st",
                                      name="bc")
                    rsl = recip[:] if qw2 == AC2 else recip[0:1, 0:qw2]
                    nc.tensor.matmul(sl(bc_ps), ones_row[:], rsl,
                                     start=True, stop=True)
                    bc_sb = smallpool.tile([128, AC2], dt.float16, tag="bc_sb")
                    nc.scalar.copy(sl(bc_sb), sl(bc_ps))
                    attnT2 = attnpool.tile([128, AC2], dt.float16, tag="attnT")
                    nc.vector.tensor_mul(sl(attnT2), sl(ot2), sl(bc_sb))
                    attnTs[key][p] = attnT2
                    done.append(key)
                pendB.clear()
                for key in dict.fromkeys(done):
                    if len(attnTs[key]) == NP:
                        push_oproj(key)

            def att_chunk(c, half=None, last=False):
                if half is None:
                    q0, qw = c * AC, AC
                    nb = 2 * (c + 1) if causal else N_ST
                    d0 = 2 * c if causal else N_ST
                else:
                    q0, qw = c * AC + half * 128, 128
                    nb = 2 * c + 1 + half
                    d0 = nb - 1
                key = (c, half)
                kmeta[key] = (q0, qw)
                attnTs[key] = {}
                qw2 = 2 * qw
                qoff = 0 if half is None else half * 128
                sl = (lambda t: t[:]) if qw2 == AC2 \
                    else (lambda t: t[:, 0:qw2])
                ots = [ps_ot.tile([128, AC2], dt.float32, tag="ot", name="ot")
                       for _ in range(NP)]
                esums = [epool.tile([128, AC2], dt.bfloat16, tag="esum",
                                    name="esum", bufs=2) for _ in range(NP)]
                for b in range(nb):
                    diag = causal and b >= d0
                    e2s = []
                    for p in range(NP):
                        st2 = ps_s.tile([128, AC2], dt.float32, tag="st",
                                        name="st")
                        nc.tensor.matmul(sl(st2),
                                         kT_sb[:, b * 128:(b + 1) * 128],
                                         qT_sb[:, c, 2 * p:2 * p + 2,
                                               qoff:qoff + qw],
                                         start=True, stop=True)
                        e2 = epool.tile([128, AC2], dt.bfloat16, tag="e",
                                        name="e2", bufs=4)
                        nc.scalar.activation(sl(e2), sl(st2),
                                             mybir.ActivationFunctionType.Exp,
                                             bias=ebias[:])
                        if diag:
                            tslot = 2 if half is not None else b - d0
                            nc.vector.tensor_mul(sl(e2), sl(e2),
                                                 tri_sb[:, tslot, 0:qw2])
                        e2s.append(e2)
                    pump(4 if last else (3 if diag else 2))
                    for p in range(NP):
                        nc.tensor.matmul(sl(ots[p]), v_sb[:, b, :],
                                         sl(e2s[p]),
                                         start=(b == 0), stop=(b == nb - 1))
                        with nc.allow_low_precision(reason="softmax denom"):
                            if b == 0:
                                nc.vector.tensor_copy(sl(esums[p]),
                                                      sl(e2s[p]))
                            else:
                                nc.vector.tensor_add(sl(esums[p]),
                                                     sl(esums[p]),
                                                     sl(e2s[p]))
                # stage A: denominator + fast reciprocal (off the PE path)
                for p in range(NP):
                    den = ps_s.tile([1, AC2], dt.float32, tag="st", name="den")
                    nc.tensor.matmul(den[0:1, 0:qw2] if qw2 != AC2 else den[:],
                                     ones_col[:], sl(esums[p]),
                                     start=True, stop=True)
                    recip = smallpool.tile([1, AC2], dt.float32, tag="recip")
                    nc.vector.reciprocal_approx_fast(
                        recip[0:1, 0:qw2] if qw2 != AC2 else recip[:],
                        den[0:1, 0:qw2] if qw2 != AC2 else den[:])
                    recip16 = smallpool.tile([1, AC2], dt.float16,
                                             tag="recip16")
                    nc.scalar.copy(
                        recip16[0:1, 0:qw2] if qw2 != AC2 else recip16[:],
                        recip[0:1, 0:qw2] if qw2 != AC2 else recip[:])
                    pendB.append((key, p, ots[p], recip16))

            # ===================== driver =====================
            push_proj(0)
            flush(proj_q)
            for c in range(N_AC):
                if c % 2 == 0 and c // 2 + 1 < N_PC:
                    push_proj(c // 2 + 1)
                stageB_flush()
                if causal and c == N_AC - 1:
                    att_chunk(c, half=0, last=True)
                    stageB_flush()
                    att_chunk(c, half=1, last=True)
                else:
                    att_chunk(c)
                if c % 2 == 1:
                    flush(proj_q)
            stageB_flush()
            flush(oproj_q)

    nc.compile()
    _BUILD_CACHE[key] = nc
    return nc


def _prep_inputs(hidden_states, attention_mask, cos, sin, Wq, Wk, Wv, Wo):
    X = np.asarray(hidden_states, dtype=np.float32).reshape(S, E)
    # [N_PC, 128, NE, PC]: exact SBUF tile layout per chunk -> long DMA runs
    XT4 = np.ascontiguousarray(
        X.reshape(N_PC, PC, NE, 128).transpose(0, 3, 2, 1)).astype(F16)

    m = np.asarray(attention_mask, dtype=np.float32).reshape(S, S)
    il, ju = np.tril_indices(S), np.triu_indices(S, 1)
    causal = bool(np.all(m[il] == 0.0) and np.all(m[ju] <= -1e8))
    dense = bool(np.all(m == 0.0))
    if not (causal or dense):
        raise NotImplementedError("only causal or all-zero masks supported")

    # the 1/sqrt(D) score scale is folded into Wq host-side
    cosT = np.ascontiguousarray(np.asarray(cos, np.float32).reshape(S, D).T)
    sinT = np.ascontiguousarray(np.asarray(sin, np.float32).reshape(S, D).T)
    sin_mod = sinT.copy()
    sin_mod[0:64] *= -1.0
    rope_t = np.stack([cosT, sin_mod]).astype(F16)

    p = np.arange(128)[:, None]
    f = np.arange(AC)[None, :]
    tri1 = np.stack([(128 * jj + p <= f) for jj in range(2)])
    tri12 = np.concatenate([tri1, tri1], axis=2)           # [2,128,2*AC]
    f128 = np.arange(128)[None, :]
    trih = np.tile((p <= f128), (1, 4))[None]              # [1,128,2*AC]
    tri = np.concatenate([tri12, trih], axis=0).astype(BF16)

    Wq = np.asarray(Wq, np.float32)
    Wk = np.asarray(Wk, np.float32)
    Wv = np.asarray(Wv, np.float32)
    Wo = np.asarray(Wo, np.float32)

    def wtile(Wshard, dtype=F16):
        # [out, E] -> SBUF layout [128, NE, out]
        return np.ascontiguousarray(
            Wshard.T.reshape(NE, 128, Wshard.shape[0]).transpose(1, 0, 2)
        ).astype(dtype)

    q_pre = 1.0 / math.sqrt(D)

    in_maps = []
    for c in range(N_CORES):
        in_maps.append({
            "xt": XT4,
            "wqt": wtile(Wq[c * OSH:(c + 1) * OSH, :] * q_pre),
            "wkt": wtile(Wk[c * D:(c + 1) * D, :]),
            "wvt": wtile(Wv[c * D:(c + 1) * D, :]),
            "wot": np.ascontiguousarray(
                Wo[:, c * OSH:(c + 1) * OSH].T.reshape(G, 128, E)
                .transpose(1, 0, 2)).astype(F16),
            "rope": rope_t,
            "tri": tri,
        })
    return in_maps, causal


def kernel(hidden_states, attention_mask, cos, sin, Wq, Wk, Wv, Wo,
           _trace=False, _tmpdir=None):
    in_maps, causal = _prep_inputs(hidden_states, attention_mask, cos, sin,
                                   Wq, Wk, Wv, Wo)
    nc = build_bass(causal)
    res = run_bass_kernel_spmd(nc, in_maps, core_ids=list(range(N_CORES)),
                               trace=_trace, tmpdir=_tmpdir)
    out = res.results[0]["out"].astype(np.float32)
    for c in range(1, N_CORES):
        out = out + res.results[c]["out"]
    kernel._last_result = res
    return out.reshape(1, S, E).astype(np.float32)
